# revision 35
# baseline (speedup 1.0000x reference)
"""Tensor-parallel single-step attention-decoder (embed + attn + GRU + tied
log-softmax head) for 8 Trainium2 NeuronCores.

Sharding: the vocab head W_o/b_o is sharded over the vocab dim (the
memory-bound bulk, streamed in bf16); attention tensors are sharded over
their contraction/output dims; the GRU gate weights are sharded over the
hidden dim. Cross-core steps: AllReduce of attention score partials,
AllGather of the context vector, AllGather of the new hidden state,
AllGather of per-core (max, sumexp) stats for the global log-softmax
normalizer.

Precision: attention score path stays fp32 (softmax amplifies score error);
GRU gate weights and the attention value path use fp32r (PE full rate, same
bytes); the W_o stream is bf16 (halves the dominant HBM traffic; measured
~1e-2 absmax on the log-softmax output, fp32-exact GRU hidden output).

Host-side work is layout only: slicing, transposition, packing, padding,
dtype casts.
"""

import numpy as np
import ml_dtypes

import concourse.bass as bass
import concourse.mybir as mybir
import concourse.tile as tile
from concourse import bacc
from concourse.bass_utils import run_bass_kernel_spmd
from concourse.masks import make_identity

V, H, E, L = 50257, 1024, 1024, 2048
NC = 8
VS = 6656           # padded vocab shard per core = 13 * 512
VP = NC * VS        # 53248
NEG = -30000.0      # pad bias: exp underflows to 0, never the max
F32 = mybir.dt.float32
F32R = mybir.dt.float32r
BF16 = mybir.dt.bfloat16
Act = mybir.ActivationFunctionType
AX = mybir.AxisListType.X
RG = [list(range(NC))]
BLOCKS = [(0, 5), (5, 5), (10, 3)]  # phase-F vocab-group rounds


def _build():
    nc = bacc.Bacc("TRN2", target_bir_lowering=False, debug=False, num_devices=NC)

    # phase-A/B critical inputs first (DMA issue order follows program order)
    d_hiT = nc.dram_tensor("hiT", [128, 8], F32, kind="ExternalInput")
    d_wt = nc.dram_tensor("wt", [128, 8, 128], F32, kind="ExternalInput")
    d_ca = nc.dram_tensor("ca", [128, L], F32, kind="ExternalInput")
    d_gT = nc.dram_tensor("gT", [128, 8, 2], F32R, kind="ExternalInput")
    d_hiT2 = nc.dram_tensor("hiT2", [128, 8, 2], F32R, kind="ExternalInput")
    d_whh = nc.dram_tensor("whh", [128, 8, 384], F32R, kind="ExternalInput")
    d_wihg = nc.dram_tensor("wihg", [128, 8, 384], F32R, kind="ExternalInput")
    d_wihc = nc.dram_tensor("wihc", [128, 8, 384], F32R, kind="ExternalInput")
    d_cct = nc.dram_tensor("cct", [128, 16, 128], F32R, kind="ExternalInput")
    d_wo = nc.dram_tensor("wo", [4, 128, 2, VS], BF16, kind="ExternalInput")
    d_btc = nc.dram_tensor("btc", [128, 1], F32, kind="ExternalInput")
    d_gc = nc.dram_tensor("gc", [128, 1], F32, kind="ExternalInput")
    d_h0s = nc.dram_tensor("h0s", [1, 128], F32, kind="ExternalInput")
    d_bih = nc.dram_tensor("bih", [1, 384], F32, kind="ExternalInput")
    d_bhh = nc.dram_tensor("bhh", [1, 384], F32, kind="ExternalInput")
    d_bo = nc.dram_tensor("bo", [128, VS // 128], F32, kind="ExternalInput")
    d_ols = nc.dram_tensor("out_ls", [128, VS // 128], F32, kind="ExternalOutput")
    d_oh = nc.dram_tensor("out_h", [1, 128], F32, kind="ExternalOutput")

    with tile.TileContext(nc) as tc:
        with (
            tc.tile_pool(name="cst", bufs=1) as cst,
            tc.tile_pool(name="wts", bufs=1) as wts,
            tc.tile_pool(name="wop", bufs=3) as wop,
            tc.tile_pool(name="wk", bufs=1) as wk,
            tc.tile_pool(name="ps", bufs=2, space="PSUM") as ps,
            tc.tile_pool(name="ps4", bufs=5, space="PSUM") as ps4,
            tc.tile_pool(name="pst", bufs=1, space="PSUM") as pst,
            tc.tile_pool(name="dram", bufs=1, space="DRAM") as dram,
        ):
            # ---- critical-path loads first
            hiT = cst.tile([128, 8], F32, name="hiT_s", tag="hiT_s")
            nc.sync.dma_start(hiT[:], d_hiT[:])
            wt = wts.tile([128, 8, 128], F32, name="wt_s", tag="wt_s")
            nc.sync.dma_start(wt[:], d_wt[:])
            ca = wts.tile([128, L], F32, name="ca_s", tag="ca_s")
            nc.sync.dma_start(ca[:], d_ca[:])

            def load(dt_, shape, nm, pool=cst, dt=F32):
                t = pool.tile(shape, dt, name=nm, tag=nm)
                nc.sync.dma_start(t[:], dt_[:])
                return t

            btc = load(d_btc, [128, 1], "btc_s")
            gc = load(d_gc, [128, 1], "gc_s")
            h0s = load(d_h0s, [1, 128], "h0s_s")
            bih = load(d_bih, [1, 384], "bih_s")
            bhh = load(d_bhh, [1, 384], "bhh_s")
            bo = load(d_bo, [128, VS // 128], "bo_s")
            gT = load(d_gT, [128, 8, 2], "gT_s", dt=F32R)
            hiTr = load(d_hiT2, [128, 8, 2], "hiTr_s", dt=F32R)
            whh = load(d_whh, [128, 8, 384], "whh_s", pool=wts, dt=F32R)
            wihg = load(d_wihg, [128, 8, 384], "wihg_s", pool=wts, dt=F32R)
            cct = load(d_cct, [128, 16, 128], "cct_s", pool=wts, dt=F32R)
            wihc = load(d_wihc, [128, 8, 384], "wihc_s", pool=wts, dt=F32R)

            # ---- constants (gpsimd/DVE, no DMA)
            ident = cst.tile([128, 128], F32, name="ident", tag="ident")
            make_identity(nc, ident[:])
            ones = cst.tile([1, 128], F32, name="ones", tag="ones")
            nc.vector.memset(ones[:], 1.0)
            nones = cst.tile([1, 128], F32, name="nones", tag="nones")
            nc.vector.memset(nones[:], -1.0)

            # ---- helpers
            def transp(src_ap, p, q, nm, dt=F32):
                """SBUF [p, q] -> SBUF [q, p] via PE transpose."""
                t = pst.tile([q, p], F32, name=f"{nm}_ps", tag="tiny_ps")
                nc.tensor.transpose(t[:], src_ap, ident[0:p, 0:p])
                o = wk.tile([q, p], dt, name=nm, tag=nm)
                nc.vector.tensor_copy(o[:], t[:])
                return o

            def bcast(val, n, neg, nm):
                """SBUF [1,1] -> SBUF [n,1] replicated (optionally negated)."""
                t = pst.tile([n, 1], F32, name=f"{nm}_ps", tag="tiny_ps")
                lhs = (nones if neg else ones)[0:1, 0:n]
                nc.tensor.matmul(t[:], lhs, val[:], start=True, stop=True)
                o = wk.tile([n, 1], F32, name=nm, tag=nm)
                nc.vector.tensor_copy(o[:], t[:])
                return o

            # ---- phase A: d^T (this core's E-slice), gx_g, gh
            dT_ps = ps.tile([128, 1], F32, name="dT_ps", tag="acc")
            for k in range(8):
                nc.tensor.matmul(
                    dT_ps[:], wt[:, k, :], hiT[:, k : k + 1],
                    start=(k == 0), stop=(k == 7),
                )
            dT = wk.tile([128, 1], F32, name="dT", tag="dT_sb")
            nc.vector.tensor_add(dT[:], dT_ps[:], btc[:])
            nc.vector.tensor_add(dT[:], dT[:], gc[:])

            # ---- phase B: partial attention scores + AllReduce
            ssb = wk.tile([1, L], F32, name="ssb", tag="ssb")
            for j in range(4):
                s_ps = ps4.tile([1, 512], F32, name=f"s_ps{j}", tag="bank512")
                nc.tensor.matmul(
                    s_ps[:], dT[:, 0:1], ca[:, 512 * j : 512 * (j + 1)],
                    start=True, stop=True,
                )
                nc.vector.tensor_copy(ssb[:, 512 * j : 512 * (j + 1)], s_ps[:])
            sc_in = dram.tile([1, L], F32, name="sc_in", tag="sc_in")
            sc_out = dram.tile([16, 128], F32, name="sc_out", tag="sc_out")
            nc.sync.dma_start(sc_in[:], ssb[:])
            nc.gpsimd.collective_compute(
                "AllReduce", mybir.AluOpType.add, replica_groups=RG,
                ins=[sc_in.opt()], outs=[sc_out.opt()],
            )

            # gx_g and gh overlap the AllReduce window
            gxg_ps = ps.tile([2, 384], F32, name="gxg_ps", tag="acc")
            for k in range(8):
                nc.tensor.matmul(
                    gxg_ps[:], gT[:, k, :], wihg[:, k, :],
                    start=(k == 0), stop=(k == 7),
                )
            gxg = wk.tile([1, 384], F32, name="gxg", tag="gxg_sb")
            nc.vector.tensor_copy(gxg[:], gxg_ps[0:1, :])
            gh_ps = ps.tile([2, 384], F32, name="gh_ps", tag="acc")
            for k in range(8):
                nc.tensor.matmul(
                    gh_ps[:], hiTr[:, k, :], whh[:, k, :],
                    start=(k == 0), stop=(k == 7),
                )
            ghv = wk.tile([1, 384], F32, name="ghv", tag="ghv_sb")
            nc.vector.tensor_copy(ghv[:], gh_ps[0:1, :])

            # ---- phase C: softmax over the full 2048 scores (replicated)
            s16 = wk.tile([16, 128], F32, name="s16", tag="s16")
            nc.sync.dma_start(s16[:], sc_out[:])
            sT = transp(s16[:], 16, 128, "sT")                  # [128, 16]
            m128 = wk.tile([128, 1], F32, name="m128", tag="m128")
            nc.vector.reduce_max(m128[:], sT[:], axis=AX)
            mrow = transp(m128[:], 128, 1, "mrow")              # [1, 128]
            m1 = wk.tile([1, 1], F32, name="m1", tag="m1")
            nc.vector.reduce_max(m1[:], mrow[:], axis=AX)
            nm128 = bcast(m1, 128, True, "nm128")
            esT = wk.tile([128, 16], F32, name="esT", tag="esT")
            sum128 = wk.tile([128, 1], F32, name="sum128", tag="sum128")
            nc.scalar.activation(esT[:], sT[:], Act.Exp, bias=nm128[:], accum_out=sum128[:])
            srow = transp(sum128[:], 128, 1, "srow")            # [1, 128]
            S1 = wk.tile([1, 1], F32, name="S1", tag="S1")
            nc.vector.reduce_sum(S1[:], srow[:], axis=AX)
            rS = wk.tile([1, 1], F32, name="rS", tag="rS")
            nc.vector.reciprocal(rS[:], S1[:])
            rs128 = bcast(rS, 128, False, "rs128")
            aT = wk.tile([128, 16, 2], F32R, name="aT", tag="aT")
            nc.vector.tensor_scalar_mul(aT[:, :, 0], esT[:], rs128[:])
            nc.vector.tensor_scalar_mul(aT[:, :, 1], esT[:], rs128[:])

            # ---- phase D: context slice c_p + AllGather + gx_c
            cT_ps = ps.tile([128, 2], F32, name="cT_ps", tag="acc")
            for k in range(16):
                nc.tensor.matmul(
                    cT_ps[:], cct[:, k, :], aT[:, k, :],
                    start=(k == 0), stop=(k == 15),
                )
            cT_sb = wk.tile([128, 1], F32, name="cT_sb", tag="cT_sb")
            nc.vector.tensor_copy(cT_sb[:], cT_ps[:, 0:1])
            c_in = dram.tile([128, 1], F32, name="c_in", tag="c_in")
            c_out = dram.tile([8, 128], F32, name="c_out", tag="c_out")
            nc.sync.dma_start(c_in[:], cT_sb[:])
            nc.gpsimd.collective_compute(
                "AllGather", mybir.AluOpType.bypass, replica_groups=RG,
                ins=[c_in.opt()], outs=[c_out.opt()],
            )
            c8 = wk.tile([8, 128], F32, name="c8", tag="c8")
            nc.sync.dma_start(c8[:], c_out[:])
            cT_t = pst.tile([128, 8], F32, name="cT_t", tag="tiny_ps")
            nc.tensor.transpose(cT_t[:], c8[:], ident[0:8, 0:8])
            cT_all = wk.tile([128, 8, 2], F32R, name="cT_all", tag="cT_all")
            nc.vector.tensor_copy(cT_all[:, :, 0], cT_t[:])
            nc.vector.tensor_copy(cT_all[:, :, 1], cT_t[:])
            gxc_ps = ps.tile([2, 384], F32, name="gxc_ps", tag="acc")
            for k in range(8):
                nc.tensor.matmul(
                    gxc_ps[:], cT_all[:, k, :], wihc[:, k, :],
                    start=(k == 0), stop=(k == 7),
                )

            # ---- phase E: GRU gates for this core's 128 hidden units
            u = wk.tile([1, 384], F32, name="u", tag="u")
            nc.vector.tensor_add(u[:], gxg[:], gxc_ps[0:1, :])
            nc.vector.tensor_add(u[:], u[:], bih[:])
            v = wk.tile([1, 384], F32, name="v", tag="v")
            nc.vector.tensor_add(v[:], ghv[:], bhh[:])
            rz = wk.tile([1, 256], F32, name="rz", tag="rz")
            nc.vector.tensor_add(rz[:], u[:, 0:256], v[:, 0:256])
            rzs = wk.tile([1, 256], F32, name="rzs", tag="rzs")
            nc.scalar.activation(rzs[:], rz[:], Act.Sigmoid)
            npre = wk.tile([1, 128], F32, name="npre", tag="npre")
            nc.vector.tensor_mul(npre[:], rzs[:, 0:128], v[:, 256:384])
            nc.vector.tensor_add(npre[:], npre[:], u[:, 256:384])
            nt = wk.tile([1, 128], F32, name="nt", tag="nt")
            nc.scalar.activation(nt[:], npre[:], Act.Tanh)
            hm = wk.tile([1, 128], F32, name="hm", tag="hm")
            nc.vector.tensor_sub(hm[:], h0s[:], nt[:])
            nc.vector.tensor_mul(hm[:], hm[:], rzs[:, 128:256])
            hnew = wk.tile([1, 128], F32, name="hnew", tag="hnew")
            nc.vector.tensor_add(hnew[:], nt[:], hm[:])
            nc.sync.dma_start(d_oh[:], hnew[:])

            h_in = dram.tile([1, 128], F32, name="h_in", tag="h_in")
            h_out = dram.tile([8, 128], F32, name="h_out", tag="h_out")
            nc.sync.dma_start(h_in[:], hnew[:])
            nc.gpsimd.collective_compute(
                "AllGather", mybir.AluOpType.bypass, replica_groups=RG,
                ins=[h_in.opt()], outs=[h_out.opt()],
            )
            h8 = wk.tile([8, 128], F32, name="h8", tag="h8")
            nc.sync.dma_start(h8[:], h_out[:])
            hnT = transp(h8[:], 8, 128, "hnT", dt=BF16)         # [128, 8] bf16

            # ---- phase F: vocab-shard logits, streamed over bf16 W_o slabs
            # PSUM matvec outputs are [1, 512] rows (one bank each, max 5 live),
            # so the 13 vocab groups stream in rounds of 5/5/3. Each DMA brings
            # a paired-k slab [128, 2, round_width].
            ls_flat = wk.tile([1, VS], F32, name="ls_flat", tag="ls_flat")
            for bi, (g0, ng) in enumerate(BLOCKS):
                Lg = [
                    ps4.tile([1, 512], F32, name=f"Lg{g0 + gi}", tag="bank512")
                    for gi in range(ng)
                ]
                for K in range(4):
                    slab = wop.tile(
                        [128, 2, 512 * ng], BF16, name=f"slab{bi}_{K}",
                        tag="slab5" if ng == 5 else "slab_t",
                    )
                    nc.sync.dma_start(
                        slab[:], d_wo[K][:, :, 512 * g0 : 512 * (g0 + ng)]
                    )
                    for j in range(2):
                        k = 2 * K + j
                        for gi in range(ng):
                            nc.tensor.matmul(
                                Lg[gi][:],
                                hnT[:, k : k + 1],
                                slab[:, j, 512 * gi : 512 * (gi + 1)],
                                start=(k == 0), stop=(k == 7),
                            )
                for gi in range(ng):
                    g = g0 + gi
                    nc.vector.tensor_copy(
                        ls_flat[:, 512 * g : 512 * (g + 1)], Lg[gi][:]
                    )

            # reshape the flat logits row across partitions: [1, 6656] -> [128, 52]
            W52 = VS // 128
            ls128r = wk.tile([128, W52], F32, name="ls128r", tag="ls128r")
            nc.sync.dma_start(ls128r[:], ls_flat[:])
            ls128 = wk.tile([128, W52], F32, name="ls128", tag="ls128")
            nc.vector.tensor_add(ls128[:], ls128r[:], bo[:])

            # ---- phase G: global log-softmax
            m52 = wk.tile([128, 1], F32, name="m52", tag="m52")
            nc.vector.reduce_max(m52[:], ls128[:], axis=AX)
            m52r = transp(m52[:], 128, 1, "m52r")               # [1, 128]
            mloc = wk.tile([1, 1], F32, name="mloc", tag="mloc")
            nc.vector.reduce_max(mloc[:], m52r[:], axis=AX)
            nm52 = bcast(mloc, 128, True, "nm52")
            scr = wk.tile([128, W52], F32, name="scr", tag="scr")
            s52 = wk.tile([128, 1], F32, name="s52", tag="s52")
            nc.scalar.activation(scr[:], ls128[:], Act.Exp, bias=nm52[:], accum_out=s52[:])
            s52r = transp(s52[:], 128, 1, "s52r")               # [1, 128]
            Sloc = wk.tile([1, 1], F32, name="Sloc", tag="Sloc")
            nc.vector.reduce_sum(Sloc[:], s52r[:], axis=AX)

            st2 = wk.tile([1, 8], F32, name="st2", tag="st2")
            nc.vector.memset(st2[:], 0.0)
            nc.vector.tensor_copy(st2[:, 0:1], mloc[:])
            nc.vector.tensor_copy(st2[:, 1:2], Sloc[:])
            st_in = dram.tile([1, 8], F32, name="st_in", tag="st_in")
            st_out = dram.tile([8, 8], F32, name="st_out", tag="st_out")
            nc.sync.dma_start(st_in[:], st2[:])
            nc.gpsimd.collective_compute(
                "AllGather", mybir.AluOpType.bypass, replica_groups=RG,
                ins=[st_in.opt()], outs=[st_out.opt()],
            )
            st8 = wk.tile([8, 8], F32, name="st8", tag="st8")
            nc.sync.dma_start(st8[:], st_out[:])
            m8 = transp(st8[:, 0:1], 8, 1, "m8")                # [1, 8]: all m_p
            S8 = transp(st8[:, 1:2], 8, 1, "S8")                # [1, 8]: all S_p
            gm = wk.tile([1, 1], F32, name="gm", tag="gm")
            nc.vector.reduce_max(gm[:], m8[:], axis=AX)
            ngm = wk.tile([1, 1], F32, name="ngm", tag="ngm")
            nc.vector.tensor_scalar_mul(ngm[:], gm[:], -1.0)
            ew = wk.tile([1, 8], F32, name="ew", tag="ew")
            nc.scalar.activation(ew[:], m8[:], Act.Exp, bias=ngm[:])
            nc.vector.tensor_mul(ew[:], ew[:], S8[:])
            Sg = wk.tile([1, 1], F32, name="Sg", tag="Sg")
            nc.vector.reduce_sum(Sg[:], ew[:], axis=AX)
            lnS = wk.tile([1, 1], F32, name="lnS", tag="lnS")
            nc.scalar.activation(lnS[:], Sg[:], Act.Ln)
            off = wk.tile([1, 1], F32, name="off", tag="off")
            nc.vector.tensor_add(off[:], gm[:], lnS[:])
            noff128 = bcast(off, 128, True, "noff128")
            outt = wk.tile([128, W52], F32, name="outt", tag="outt")
            nc.scalar.activation(outt[:], ls128[:], Act.Identity, bias=noff128[:])
            nc.sync.dma_start(d_ols[:], outt[:])

    nc.compile()
    return nc


_CACHE = {}


def _get_nc():
    if "nc" not in _CACHE:
        _CACHE["nc"] = _build()
    return _CACHE["nc"]


def _prep_in_maps(y_i, h_i, cnn_a, cnn_c, emb, W_t, b_t, W_ih, W_hh, b_ih, b_hh, W_o, b_o):
    f = np.float32
    y = int(np.asarray(y_i).reshape(-1)[0])
    g = np.ascontiguousarray(np.asarray(emb)[y], dtype=f)          # [1024]
    hi = np.ascontiguousarray(np.asarray(h_i, dtype=f).reshape(H))
    hiT = np.ascontiguousarray(hi.reshape(8, 128).T)
    gT = np.ascontiguousarray(g.reshape(8, 128).T)
    WtT = np.asarray(W_t, dtype=f).T
    WihT = np.asarray(W_ih, dtype=f).T                              # [2048, 3072]
    WhhT = np.asarray(W_hh, dtype=f).T                              # [1024, 3072]
    ca_f = np.asarray(cnn_a, dtype=f)[0]                            # [1024, 2048]
    cctf = np.asarray(cnn_c, dtype=f)[0].T                          # [2048, 1024]
    b_t = np.asarray(b_t, dtype=f)
    b_ih = np.asarray(b_ih, dtype=f)
    b_hh = np.asarray(b_hh, dtype=f)
    Wo = np.asarray(W_o, dtype=f)
    bo_pad = np.full(VP, NEG, f)
    bo_pad[:V] = np.asarray(b_o, dtype=f)

    in_maps = []
    for p in range(NC):
        sl = slice(128 * p, 128 * (p + 1))
        cols = np.r_[
            128 * p : 128 * (p + 1),
            H + 128 * p : H + 128 * (p + 1),
            2 * H + 128 * p : 2 * H + 128 * (p + 1),
        ]
        wt_p = np.ascontiguousarray(
            np.ascontiguousarray(WtT[:, sl]).reshape(8, 128, 128).transpose(1, 0, 2)
        )
        wih_p = np.ascontiguousarray(WihT[:, cols]).reshape(16, 128, 384)
        wihg_p = np.ascontiguousarray(wih_p[:8].transpose(1, 0, 2))
        wihc_p = np.ascontiguousarray(wih_p[8:].transpose(1, 0, 2))
        whh_p = np.ascontiguousarray(
            np.ascontiguousarray(WhhT[:, cols]).reshape(8, 128, 384).transpose(1, 0, 2)
        )
        cct_p = np.ascontiguousarray(
            np.ascontiguousarray(cctf[:, sl]).reshape(16, 128, 128).transpose(1, 0, 2)
        )
        v0, v1 = VS * p, min(V, VS * (p + 1))
        wo_p = np.zeros((H, VS), f)
        if v1 > v0:
            wo_p[:, : v1 - v0] = Wo[v0:v1].T
        # [H, VS] -> paired-k interleave [4, 128, 2, VS], cast bf16
        wo_b = np.ascontiguousarray(
            wo_p.reshape(4, 2, 128, VS).transpose(0, 2, 1, 3)
        ).astype(ml_dtypes.bfloat16)
        in_maps.append({
            "hiT": hiT,
            "hiT2": np.ascontiguousarray(np.repeat(hiT[:, :, None], 2, axis=2)),
            "gT": np.ascontiguousarray(np.repeat(gT[:, :, None], 2, axis=2)),
            "wt": wt_p,
            "wihg": wihg_p,
            "wihc": wihc_p,
            "whh": whh_p,
            "ca": np.ascontiguousarray(ca_f[sl]),
            "cct": cct_p,
            "wo": wo_b,
            "btc": np.ascontiguousarray(b_t[sl]).reshape(128, 1),
            "gc": np.ascontiguousarray(g[sl]).reshape(128, 1),
            "h0s": np.ascontiguousarray(hi[sl]).reshape(1, 128),
            "bih": np.ascontiguousarray(b_ih[cols]).reshape(1, 384),
            "bhh": np.ascontiguousarray(b_hh[cols]).reshape(1, 384),
            "bo": np.ascontiguousarray(bo_pad[VS * p : VS * (p + 1)]).reshape(128, VS // 128),
        })
    return in_maps


def kernel(y_i, h_i, cnn_a, cnn_c, emb, W_t, b_t, W_ih, W_hh, b_ih, b_hh, W_o, b_o):
    in_maps = _prep_in_maps(
        y_i, h_i, cnn_a, cnn_c, emb, W_t, b_t, W_ih, W_hh, b_ih, b_hh, W_o, b_o
    )
    nc = _get_nc()
    res = run_bass_kernel_spmd(nc, in_maps, core_ids=list(range(NC)))
    ls = np.concatenate([r["out_ls"].reshape(-1) for r in res.results])[:V]
    gh = np.concatenate([r["out_h"].reshape(-1) for r in res.results])
    return ls.reshape(1, V), gh.reshape(1, 1, H)


# revision 36
# speedup vs baseline: 1.1365x; 1.1365x over previous
"""Tensor-parallel single-step attention-decoder (embed + attn + GRU + tied
log-softmax head) for 8 Trainium2 NeuronCores.

Sharding: the vocab head W_o/b_o is sharded over the vocab dim (the
memory-bound bulk, streamed in bf16); attention tensors are sharded over
their contraction/output dims; the GRU gate weights are sharded over the
hidden dim. Cross-core steps: AllReduce of attention score partials,
AllGather of the context vector, AllGather of the new hidden state,
AllGather of per-core (max, sumexp) stats for the global log-softmax
normalizer.

Precision: attention score path stays fp32 (softmax amplifies score error);
GRU gate weights and the attention value path use fp32r (PE full rate, same
bytes); the W_o stream is bf16 (halves the dominant HBM traffic; measured
~1e-2 absmax on the log-softmax output vs the fp32 reference).

Host-side work is layout only: slicing, transposition, packing, padding,
dtype casts. Small inputs are packed into three SBUF-image tensors so the
whole prologue is three large DMAs.
"""

import numpy as np
import ml_dtypes

import concourse.bass as bass
import concourse.mybir as mybir
import concourse.tile as tile
from concourse import bacc
from concourse.bass_utils import run_bass_kernel_spmd
from concourse.masks import make_identity

V, H, E, L = 50257, 1024, 1024, 2048
NC = 8
VS = 6656           # padded vocab shard per core = 13 * 512
VP = NC * VS        # 53248
NEG = -30000.0      # pad bias: exp underflows to 0, never the max
F32 = mybir.dt.float32
F32R = mybir.dt.float32r
BF16 = mybir.dt.bfloat16
Act = mybir.ActivationFunctionType
AX = mybir.AxisListType.X
RG = [list(range(NC))]
BLOCKS = [(0, 5), (5, 5), (10, 3)]  # phase-F vocab-group rounds

# column layout of the packed "smalls" image [128, SM_W] (f32r-typed; fp32
# regions are bitcast views)
SM_HIT2 = 0          # [128, 8, 2] f32r  (cols 0:16)
SM_GT2 = 16          # [128, 8, 2] f32r  (cols 16:32)
SM_HIT = 32          # [128, 8]   f32    (cols 32:40)
SM_BTC = 40          # [128, 1]   f32
SM_GC = 41           # [128, 1]   f32
SM_BO = 42           # [128, 52]  f32    (cols 42:94)
SM_H0S = 94          # [1, 128]   f32 row
SM_BIH = 222         # [1, 384]   f32 row
SM_BHH = 606         # [1, 384]   f32 row
SM_W = 992

# packed fp32 weight image [128, 3072]: wt pack | ca
WA_WT = 0            # [128, 8, 128]
WA_CA = 1024         # [128, 2048]
WA_W = 3072

# packed fp32r weight image [128, 11264]: whh | wihg | cct | wihc
WB_WHH = 0           # [128, 8, 384]
WB_WIHG = 3072       # [128, 8, 384]
WB_CCT = 6144        # [128, 16, 128]
WB_WIHC = 8192       # [128, 8, 384]
WB_W = 11264


def _build():
    nc = bacc.Bacc("TRN2", target_bir_lowering=False, debug=False, num_devices=NC)

    d_sm = nc.dram_tensor("sm", [128, SM_W], F32R, kind="ExternalInput")
    d_wa = nc.dram_tensor("wa", [128, WA_W], F32, kind="ExternalInput")
    d_wb = nc.dram_tensor("wb", [128, WB_W], F32R, kind="ExternalInput")
    d_wo = nc.dram_tensor("wo", [4, 128, 2, VS], BF16, kind="ExternalInput")
    d_ols = nc.dram_tensor("out_ls", [128, VS // 128], F32, kind="ExternalOutput")
    d_oh = nc.dram_tensor("out_h", [1, 128], F32, kind="ExternalOutput")

    with tile.TileContext(nc) as tc:
        with (
            tc.tile_pool(name="cst", bufs=1) as cst,
            tc.tile_pool(name="wts", bufs=1) as wts,
            tc.tile_pool(name="wop", bufs=3) as wop,
            tc.tile_pool(name="wk", bufs=1) as wk,
            tc.tile_pool(name="ps", bufs=2, space="PSUM") as ps,
            tc.tile_pool(name="ps4", bufs=5, space="PSUM") as ps4,
            tc.tile_pool(name="pst", bufs=1, space="PSUM") as pst,
            tc.tile_pool(name="dram", bufs=1, space="DRAM") as dram,
        ):
            # ---- three packed prologue DMAs (critical first)
            wa = wts.tile([128, WA_W], F32, name="wa_s", tag="wa_s")
            nc.sync.dma_start(wa[:], d_wa[:])
            sm = cst.tile([128, SM_W], F32R, name="sm_s", tag="sm_s")
            nc.sync.dma_start(sm[:], d_sm[:])
            wb = wts.tile([128, WB_W], F32R, name="wb_s", tag="wb_s")
            nc.sync.dma_start(wb[:], d_wb[:])

            # slice views into the packed images
            wt = wa[:, WA_WT:WA_CA].rearrange("p (k n) -> p k n", k=8)
            ca = wa[:, WA_CA:WA_W]
            hiT2 = sm[:, SM_HIT2:SM_GT2].rearrange("p (k j) -> p k j", j=2)
            gT2 = sm[:, SM_GT2:SM_HIT].rearrange("p (k j) -> p k j", j=2)
            hiT = sm[:, SM_HIT : SM_HIT + 8].bitcast(F32)
            btc = sm[:, SM_BTC : SM_BTC + 1].bitcast(F32)
            gc = sm[:, SM_GC : SM_GC + 1].bitcast(F32)
            bo = sm[:, SM_BO : SM_BO + 52].bitcast(F32)
            h0s = sm[0:1, SM_H0S : SM_H0S + 128].bitcast(F32)
            bih = sm[0:1, SM_BIH : SM_BIH + 384].bitcast(F32)
            bhh = sm[0:1, SM_BHH : SM_BHH + 384].bitcast(F32)
            whh = wb[:, WB_WHH:WB_WIHG].rearrange("p (k n) -> p k n", k=8)
            wihg = wb[:, WB_WIHG:WB_CCT].rearrange("p (k n) -> p k n", k=8)
            cct = wb[:, WB_CCT:WB_WIHC].rearrange("p (k n) -> p k n", k=16)
            wihc = wb[:, WB_WIHC:WB_W].rearrange("p (k n) -> p k n", k=8)

            # ---- constants (gpsimd/DVE, no DMA)
            ident = cst.tile([128, 128], F32, name="ident", tag="ident")
            make_identity(nc, ident[:])
            ones = cst.tile([1, 128], F32, name="ones", tag="ones")
            nc.vector.memset(ones[:], 1.0)
            nones = cst.tile([1, 128], F32, name="nones", tag="nones")
            nc.vector.memset(nones[:], -1.0)

            # pre-warm ACT LUT sets during the DMA window
            warm = cst.tile([1, 4], F32, name="warm", tag="warm")
            nc.vector.memset(warm[:], 1.0)
            for i, fn in enumerate((Act.Exp, Act.Sigmoid, Act.Tanh, Act.Ln)):
                nc.scalar.activation(warm[:, i : i + 1], warm[:, i : i + 1], fn)

            # ---- helpers
            def transp(src_ap, p, q, nm, dt=F32):
                """SBUF [p, q] -> SBUF [q, p] via PE transpose."""
                t = pst.tile([q, p], F32, name=f"{nm}_ps", tag="tiny_ps")
                nc.tensor.transpose(t[:], src_ap, ident[0:p, 0:p])
                o = wk.tile([q, p], dt, name=nm, tag=nm)
                nc.vector.tensor_copy(o[:], t[:])
                return o

            def bcast(val, n, neg, nm):
                """SBUF [1,1] -> SBUF [n,1] replicated (optionally negated)."""
                t = pst.tile([n, 1], F32, name=f"{nm}_ps", tag="tiny_ps")
                lhs = (nones if neg else ones)[0:1, 0:n]
                nc.tensor.matmul(t[:], lhs, val[:], start=True, stop=True)
                o = wk.tile([n, 1], F32, name=nm, tag=nm)
                nc.vector.tensor_copy(o[:], t[:])
                return o

            # ---- phase A: d row for this core's E-slice, then transpose
            d_ps = ps.tile([1, 128], F32, name="d_ps", tag="acc")
            for k in range(8):
                nc.tensor.matmul(
                    d_ps[:], hiT[:, k : k + 1], wt[:, k, :],
                    start=(k == 0), stop=(k == 7),
                )
            drow = wk.tile([1, 128], F32, name="drow", tag="drow")
            nc.vector.tensor_copy(drow[:], d_ps[:])
            dcol = pst.tile([128, 1], F32, name="dcol", tag="tiny_ps")
            nc.tensor.transpose(dcol[:], drow[:], ident[0:1, 0:1])
            dT = wk.tile([128, 1], F32, name="dT", tag="dT_sb")
            nc.vector.tensor_add(dT[:], dcol[:], btc)
            nc.vector.tensor_add(dT[:], dT[:], gc)

            # ---- phase B: partial attention scores + AllReduce
            ssb = wk.tile([1, L], F32, name="ssb", tag="ssb")
            for j in range(4):
                s_ps = ps4.tile([1, 512], F32, name=f"s_ps{j}", tag="bank512")
                nc.tensor.matmul(
                    s_ps[:], dT[:, 0:1], ca[:, 512 * j : 512 * (j + 1)],
                    start=True, stop=True,
                )
                dst = ssb[:, 512 * j : 512 * (j + 1)]
                if j % 2 == 0:
                    nc.vector.tensor_copy(dst, s_ps[:])
                else:
                    nc.scalar.copy(dst, s_ps[:])
            sc_in = dram.tile([1, L], F32, name="sc_in", tag="sc_in")
            sc_out = dram.tile([16, 128], F32, name="sc_out", tag="sc_out")
            nc.sync.dma_start(sc_in[:], ssb[:])
            nc.gpsimd.collective_compute(
                "AllReduce", mybir.AluOpType.add, replica_groups=RG,
                ins=[sc_in.opt()], outs=[sc_out.opt()],
            )

            # gx_g and gh overlap the AllReduce window
            gxg_ps = ps.tile([2, 384], F32, name="gxg_ps", tag="acc")
            for k in range(8):
                nc.tensor.matmul(
                    gxg_ps[:], gT2[:, k, :], wihg[:, k, :],
                    start=(k == 0), stop=(k == 7),
                )
            gxg = wk.tile([1, 384], F32, name="gxg", tag="gxg_sb")
            nc.vector.tensor_copy(gxg[:], gxg_ps[0:1, :])
            gh_ps = ps.tile([2, 384], F32, name="gh_ps", tag="acc")
            for k in range(8):
                nc.tensor.matmul(
                    gh_ps[:], hiT2[:, k, :], whh[:, k, :],
                    start=(k == 0), stop=(k == 7),
                )
            ghv = wk.tile([1, 384], F32, name="ghv", tag="ghv_sb")
            nc.vector.tensor_copy(ghv[:], gh_ps[0:1, :])

            # ---- phase C: softmax over the full 2048 scores (replicated)
            s16 = wk.tile([16, 128], F32, name="s16", tag="s16")
            nc.sync.dma_start(s16[:], sc_out[:])
            sT = transp(s16[:], 16, 128, "sT")                  # [128, 16]
            m128 = wk.tile([128, 1], F32, name="m128", tag="m128")
            nc.vector.reduce_max(m128[:], sT[:], axis=AX)
            mrow = transp(m128[:], 128, 1, "mrow")              # [1, 128]
            m1 = wk.tile([1, 1], F32, name="m1", tag="m1")
            nc.vector.reduce_max(m1[:], mrow[:], axis=AX)
            nm128 = bcast(m1, 128, True, "nm128")
            esT = wk.tile([128, 16], F32, name="esT", tag="esT")
            sum128 = wk.tile([128, 1], F32, name="sum128", tag="sum128")
            nc.scalar.activation(esT[:], sT[:], Act.Exp, bias=nm128[:], accum_out=sum128[:])
            srow = transp(sum128[:], 128, 1, "srow")            # [1, 128]
            S1 = wk.tile([1, 1], F32, name="S1", tag="S1")
            nc.vector.reduce_sum(S1[:], srow[:], axis=AX)
            rS = wk.tile([1, 1], F32, name="rS", tag="rS")
            nc.vector.reciprocal(rS[:], S1[:])
            rs128 = bcast(rS, 128, False, "rs128")
            aT = wk.tile([128, 16, 2], F32R, name="aT", tag="aT")
            nc.vector.tensor_scalar_mul(aT[:, :, 0], esT[:], rs128[:])
            nc.vector.tensor_scalar_mul(aT[:, :, 1], esT[:], rs128[:])

            # ---- phase D: context slice c_p + AllGather + gx_c
            cT_ps = ps.tile([128, 2], F32, name="cT_ps", tag="acc")
            for k in range(16):
                nc.tensor.matmul(
                    cT_ps[:], cct[:, k, :], aT[:, k, :],
                    start=(k == 0), stop=(k == 15),
                )
            cT_sb = wk.tile([128, 1], F32, name="cT_sb", tag="cT_sb")
            nc.vector.tensor_copy(cT_sb[:], cT_ps[:, 0:1])
            c_in = dram.tile([128, 1], F32, name="c_in", tag="c_in")
            c_out = dram.tile([8, 128], F32, name="c_out", tag="c_out")
            nc.sync.dma_start(c_in[:], cT_sb[:])
            nc.gpsimd.collective_compute(
                "AllGather", mybir.AluOpType.bypass, replica_groups=RG,
                ins=[c_in.opt()], outs=[c_out.opt()],
            )
            c8 = wk.tile([8, 128], F32, name="c8", tag="c8")
            nc.sync.dma_start(c8[:], c_out[:])
            cT_t = pst.tile([128, 8], F32, name="cT_t", tag="tiny_ps")
            nc.tensor.transpose(cT_t[:], c8[:], ident[0:8, 0:8])
            cT_all = wk.tile([128, 8, 2], F32R, name="cT_all", tag="cT_all")
            nc.vector.tensor_copy(cT_all[:, :, 0], cT_t[:])
            nc.vector.tensor_copy(cT_all[:, :, 1], cT_t[:])
            gxc_ps = ps.tile([2, 384], F32, name="gxc_ps", tag="acc")
            for k in range(8):
                nc.tensor.matmul(
                    gxc_ps[:], cT_all[:, k, :], wihc[:, k, :],
                    start=(k == 0), stop=(k == 7),
                )

            # ---- phase E: GRU gates for this core's 128 hidden units
            u = wk.tile([1, 384], F32, name="u", tag="u")
            nc.vector.tensor_add(u[:], gxg[:], gxc_ps[0:1, :])
            nc.vector.tensor_add(u[:], u[:], bih)
            v = wk.tile([1, 384], F32, name="v", tag="v")
            nc.vector.tensor_add(v[:], ghv[:], bhh)
            rz = wk.tile([1, 256], F32, name="rz", tag="rz")
            nc.vector.tensor_add(rz[:], u[:, 0:256], v[:, 0:256])
            rzs = wk.tile([1, 256], F32, name="rzs", tag="rzs")
            nc.scalar.activation(rzs[:], rz[:], Act.Sigmoid)
            npre = wk.tile([1, 128], F32, name="npre", tag="npre")
            nc.vector.tensor_mul(npre[:], rzs[:, 0:128], v[:, 256:384])
            nc.vector.tensor_add(npre[:], npre[:], u[:, 256:384])
            nt = wk.tile([1, 128], F32, name="nt", tag="nt")
            nc.scalar.activation(nt[:], npre[:], Act.Tanh)
            hm = wk.tile([1, 128], F32, name="hm", tag="hm")
            nc.vector.tensor_sub(hm[:], h0s, nt[:])
            nc.vector.tensor_mul(hm[:], hm[:], rzs[:, 128:256])
            hnew = wk.tile([1, 128], F32, name="hnew", tag="hnew")
            nc.vector.tensor_add(hnew[:], nt[:], hm[:])
            nc.sync.dma_start(d_oh[:], hnew[:])

            h_in = dram.tile([1, 128], F32, name="h_in", tag="h_in")
            h_out = dram.tile([8, 128], F32, name="h_out", tag="h_out")
            nc.sync.dma_start(h_in[:], hnew[:])
            nc.gpsimd.collective_compute(
                "AllGather", mybir.AluOpType.bypass, replica_groups=RG,
                ins=[h_in.opt()], outs=[h_out.opt()],
            )
            h8 = wk.tile([8, 128], F32, name="h8", tag="h8")
            nc.sync.dma_start(h8[:], h_out[:])
            hnT = transp(h8[:], 8, 128, "hnT", dt=BF16)         # [128, 8] bf16

            # ---- phase F: vocab-shard logits, streamed over bf16 W_o slabs
            # PSUM matvec outputs are [1, 512] rows (one bank each, max 5 live),
            # so the 13 vocab groups stream in rounds of 5/5/3. Each DMA brings
            # a paired-k slab [128, 2, round_width].
            ls_flat = wk.tile([1, VS], F32, name="ls_flat", tag="ls_flat")
            for bi, (g0, ng) in enumerate(BLOCKS):
                Lg = [
                    ps4.tile([1, 512], F32, name=f"Lg{g0 + gi}", tag="bank512")
                    for gi in range(ng)
                ]
                for K in range(4):
                    slab = wop.tile(
                        [128, 2, 512 * ng], BF16, name=f"slab{bi}_{K}",
                        tag="slab5" if ng == 5 else "slab_t",
                    )
                    nc.sync.dma_start(
                        slab[:], d_wo[K][:, :, 512 * g0 : 512 * (g0 + ng)]
                    )
                    for j in range(2):
                        k = 2 * K + j
                        for gi in range(ng):
                            nc.tensor.matmul(
                                Lg[gi][:],
                                hnT[:, k : k + 1],
                                slab[:, j, 512 * gi : 512 * (gi + 1)],
                                start=(k == 0), stop=(k == 7),
                            )
                for gi in range(ng):
                    g = g0 + gi
                    dst = ls_flat[:, 512 * g : 512 * (g + 1)]
                    if gi % 2 == 0:
                        nc.vector.tensor_copy(dst, Lg[gi][:])
                    else:
                        nc.scalar.copy(dst, Lg[gi][:])

            # reshape the flat logits row across partitions: [1, 6656] -> [128, 52]
            W52 = VS // 128
            ls128r = wk.tile([128, W52], F32, name="ls128r", tag="ls128r")
            nc.sync.dma_start(ls128r[:], ls_flat[:])
            ls128 = wk.tile([128, W52], F32, name="ls128", tag="ls128")
            nc.vector.tensor_add(ls128[:], ls128r[:], bo)

            # ---- phase G: global log-softmax
            m52 = wk.tile([128, 1], F32, name="m52", tag="m52")
            nc.vector.reduce_max(m52[:], ls128[:], axis=AX)
            m52r = transp(m52[:], 128, 1, "m52r")               # [1, 128]
            mloc = wk.tile([1, 1], F32, name="mloc", tag="mloc")
            nc.vector.reduce_max(mloc[:], m52r[:], axis=AX)
            nm52 = bcast(mloc, 128, True, "nm52")
            scr = wk.tile([128, W52], F32, name="scr", tag="scr")
            s52 = wk.tile([128, 1], F32, name="s52", tag="s52")
            nc.scalar.activation(scr[:], ls128[:], Act.Exp, bias=nm52[:], accum_out=s52[:])
            s52r = transp(s52[:], 128, 1, "s52r")               # [1, 128]
            Sloc = wk.tile([1, 1], F32, name="Sloc", tag="Sloc")
            nc.vector.reduce_sum(Sloc[:], s52r[:], axis=AX)

            st2 = wk.tile([1, 8], F32, name="st2", tag="st2")
            nc.vector.memset(st2[:], 0.0)
            nc.vector.tensor_copy(st2[:, 0:1], mloc[:])
            nc.vector.tensor_copy(st2[:, 1:2], Sloc[:])
            st_in = dram.tile([1, 8], F32, name="st_in", tag="st_in")
            st_out = dram.tile([8, 8], F32, name="st_out", tag="st_out")
            nc.sync.dma_start(st_in[:], st2[:])
            nc.gpsimd.collective_compute(
                "AllGather", mybir.AluOpType.bypass, replica_groups=RG,
                ins=[st_in.opt()], outs=[st_out.opt()],
            )
            st8 = wk.tile([8, 8], F32, name="st8", tag="st8")
            nc.sync.dma_start(st8[:], st_out[:])
            m8 = transp(st8[:, 0:1], 8, 1, "m8")                # [1, 8]: all m_p
            S8 = transp(st8[:, 1:2], 8, 1, "S8")                # [1, 8]: all S_p
            gm = wk.tile([1, 1], F32, name="gm", tag="gm")
            nc.vector.reduce_max(gm[:], m8[:], axis=AX)
            ngm = wk.tile([1, 1], F32, name="ngm", tag="ngm")
            nc.vector.tensor_scalar_mul(ngm[:], gm[:], -1.0)
            ew = wk.tile([1, 8], F32, name="ew", tag="ew")
            nc.scalar.activation(ew[:], m8[:], Act.Exp, bias=ngm[:])
            nc.vector.tensor_mul(ew[:], ew[:], S8[:])
            Sg = wk.tile([1, 1], F32, name="Sg", tag="Sg")
            nc.vector.reduce_sum(Sg[:], ew[:], axis=AX)
            lnS = wk.tile([1, 1], F32, name="lnS", tag="lnS")
            nc.scalar.activation(lnS[:], Sg[:], Act.Ln)
            off = wk.tile([1, 1], F32, name="off", tag="off")
            nc.vector.tensor_add(off[:], gm[:], lnS[:])
            noff128 = bcast(off, 128, True, "noff128")
            outt = wk.tile([128, W52], F32, name="outt", tag="outt")
            nc.vector.tensor_scalar_add(outt[:], ls128[:], noff128[:])
            nc.sync.dma_start(d_ols[:], outt[:])

    nc.compile()
    return nc


_CACHE = {}


def _get_nc():
    if "nc" not in _CACHE:
        _CACHE["nc"] = _build()
    return _CACHE["nc"]


def _prep_in_maps(y_i, h_i, cnn_a, cnn_c, emb, W_t, b_t, W_ih, W_hh, b_ih, b_hh, W_o, b_o):
    f = np.float32
    y = int(np.asarray(y_i).reshape(-1)[0])
    g = np.ascontiguousarray(np.asarray(emb)[y], dtype=f)          # [1024]
    hi = np.ascontiguousarray(np.asarray(h_i, dtype=f).reshape(H))
    hiT = np.ascontiguousarray(hi.reshape(8, 128).T)               # [128, 8]
    gT = np.ascontiguousarray(g.reshape(8, 128).T)
    WtT = np.asarray(W_t, dtype=f).T
    WihT = np.asarray(W_ih, dtype=f).T                              # [2048, 3072]
    WhhT = np.asarray(W_hh, dtype=f).T                              # [1024, 3072]
    ca_f = np.asarray(cnn_a, dtype=f)[0]                            # [1024, 2048]
    cctf = np.asarray(cnn_c, dtype=f)[0].T                          # [2048, 1024]
    b_t = np.asarray(b_t, dtype=f)
    b_ih = np.asarray(b_ih, dtype=f)
    b_hh = np.asarray(b_hh, dtype=f)
    Wo = np.asarray(W_o, dtype=f)
    bo_pad = np.full(VP, NEG, f)
    bo_pad[:V] = np.asarray(b_o, dtype=f)

    in_maps = []
    for p in range(NC):
        sl = slice(128 * p, 128 * (p + 1))
        cols = np.r_[
            128 * p : 128 * (p + 1),
            H + 128 * p : H + 128 * (p + 1),
            2 * H + 128 * p : 2 * H + 128 * (p + 1),
        ]
        wt_p = np.ascontiguousarray(WtT[:, sl]).reshape(8, 128, 128).transpose(1, 0, 2)
        wih_p = np.ascontiguousarray(WihT[:, cols]).reshape(16, 128, 384)
        wihg_p = wih_p[:8].transpose(1, 0, 2)
        wihc_p = wih_p[8:].transpose(1, 0, 2)
        whh_p = np.ascontiguousarray(WhhT[:, cols]).reshape(8, 128, 384).transpose(1, 0, 2)
        cct_p = np.ascontiguousarray(cctf[:, sl]).reshape(16, 128, 128).transpose(1, 0, 2)

        sm = np.zeros((128, SM_W), f)
        sm[:, SM_HIT2:SM_GT2] = np.repeat(hiT[:, :, None], 2, axis=2).reshape(128, 16)
        sm[:, SM_GT2:SM_HIT] = np.repeat(gT[:, :, None], 2, axis=2).reshape(128, 16)
        sm[:, SM_HIT : SM_HIT + 8] = hiT
        sm[:, SM_BTC] = b_t[sl]
        sm[:, SM_GC] = g[sl]
        sm[:, SM_BO : SM_BO + 52] = bo_pad[VS * p : VS * (p + 1)].reshape(128, 52)
        sm[0, SM_H0S : SM_H0S + 128] = hi[sl]
        sm[0, SM_BIH : SM_BIH + 384] = b_ih[cols]
        sm[0, SM_BHH : SM_BHH + 384] = b_hh[cols]

        wa = np.zeros((128, WA_W), f)
        wa[:, WA_WT:WA_CA] = wt_p.reshape(128, 1024)
        wa[:, WA_CA:WA_W] = ca_f[sl]

        wbuf = np.zeros((128, WB_W), f)
        wbuf[:, WB_WHH:WB_WIHG] = whh_p.reshape(128, 3072)
        wbuf[:, WB_WIHG:WB_CCT] = wihg_p.reshape(128, 3072)
        wbuf[:, WB_CCT:WB_WIHC] = cct_p.reshape(128, 2048)
        wbuf[:, WB_WIHC:WB_W] = wihc_p.reshape(128, 3072)

        v0, v1 = VS * p, min(V, VS * (p + 1))
        wo_p = np.zeros((H, VS), f)
        if v1 > v0:
            wo_p[:, : v1 - v0] = Wo[v0:v1].T
        wo_b = np.ascontiguousarray(
            wo_p.reshape(4, 2, 128, VS).transpose(0, 2, 1, 3)
        ).astype(ml_dtypes.bfloat16)

        in_maps.append({"sm": sm, "wa": wa, "wb": wbuf, "wo": wo_b})
    return in_maps


def kernel(y_i, h_i, cnn_a, cnn_c, emb, W_t, b_t, W_ih, W_hh, b_ih, b_hh, W_o, b_o):
    in_maps = _prep_in_maps(
        y_i, h_i, cnn_a, cnn_c, emb, W_t, b_t, W_ih, W_hh, b_ih, b_hh, W_o, b_o
    )
    nc = _get_nc()
    res = run_bass_kernel_spmd(nc, in_maps, core_ids=list(range(NC)))
    ls = np.concatenate([r["out_ls"].reshape(-1) for r in res.results])[:V]
    gh = np.concatenate([r["out_h"].reshape(-1) for r in res.results])
    return ls.reshape(1, V), gh.reshape(1, 1, H)
